# revision 4
# baseline (speedup 1.0000x reference)
"""Trainium2 Bass kernel for nn_CNN_CharEmb.

Computation: character embeddings -> pointwise conv (per-position linear) ->
ragged per-word max-pool over the 7 chars of each word:

  out[b, w, :] = max_{k=0..6} ( emb[x[b, 8w+k]] @ conv_w.T + conv_b )

Device strategy (8 NeuronCores, batch-sharded, 4 rows/core):
  1. Fused table M' = emb @ conv_w.T + conv_b  [70, 300] built on device by
     one matmul (the ones-row/bias-row augmentation folds the bias in), so
     h[pos] = M'[x[pos]] and the embedding+conv collapse into a row-select.
  2. The row-select is a one-hot matmul: a single DVE is_equal against an
     iota turns x (broadcast across 70 partitions) into onehot [70, L] bf16;
     then h_k tile = onehot_slice.T @ M' on the PE (no per-index DMA work).
  3. Per word-tile (128 words), 7 matmuls (char slots k=0..6, stride-8
     column slices of the onehot) land in 7 PSUM banks; a max tree split
     across DVE/ACT/GPSIMD reduces them and the f32 result DMAs out.

`wordidx` is the fixed 7-chars+boundary pattern of the reference setup;
anything else falls back to an exact host computation.
"""

import numpy as np
import ml_dtypes

import concourse.bacc as bacc
import concourse.mybir as mybir
import concourse.tile as tile
from concourse import bass_utils

# Problem shape (hardcoded per contract)
B = 32
WORD_LEN = 7
NUM_WORDS = 400
STRIDE = WORD_LEN + 1            # 8
L = NUM_WORDS * STRIDE           # 3200
EMB = 100
OUT = 300
VOCAB = 70

N_CORES = 8
B_CORE = B // N_CORES            # 4 batch rows per core
NW = B_CORE * NUM_WORDS          # 1600 words per core
LC = B_CORE * L                  # 12800 positions per core
N_TILES = (NW + 127) // 128      # 13 word-tiles (last one 64 words)
KDIM = EMB + 1                   # 101: emb + ones/bias row

BF16 = mybir.dt.bfloat16
F32 = mybir.dt.float32

LAST_RESULTS = None  # stashed BassKernelResults for the test harness


def _build_program():
    nc = bacc.Bacc("TRN2", target_bir_lowering=False, debug=False,
                   num_devices=N_CORES)

    xb_dram = nc.dram_tensor("xb", [VOCAB, LC], BF16, kind="ExternalInput")
    iota_dram = nc.dram_tensor("iota", [VOCAB, 1], F32, kind="ExternalInput")
    embT_dram = nc.dram_tensor("embT_aug", [KDIM, VOCAB], BF16,
                               kind="ExternalInput")
    wt_dram = nc.dram_tensor("wt_aug", [KDIM, OUT], BF16, kind="ExternalInput")
    out_dram = nc.dram_tensor("out", [NW, OUT], F32, kind="ExternalOutput")

    with tile.TileContext(nc) as tc:
        with (
            tc.tile_pool(name="const", bufs=1) as cpool,
            tc.tile_pool(name="oh", bufs=1) as ohpool,
            tc.tile_pool(name="work", bufs=3) as wpool,
            tc.tile_pool(name="pmain", bufs=1, space="PSUM") as pmain,
            tc.tile_pool(name="paux", bufs=2, space="PSUM") as paux,
        ):
            embT_t = cpool.tile([KDIM, VOCAB], BF16)
            wt_t = cpool.tile([KDIM, OUT], BF16)
            iota_t = cpool.tile([VOCAB, 1], F32)
            xb_t = cpool.tile([VOCAB, LC], BF16)
            nc.sync.dma_start(embT_t[:], embT_dram[:])
            nc.sync.dma_start(wt_t[:], wt_dram[:])
            nc.sync.dma_start(iota_t[:], iota_dram[:])
            nc.sync.dma_start(xb_t[:], xb_dram[:])

            # Fused table M' = emb @ W.T + b   [70, 300] bf16
            mp_ps = paux.tile([VOCAB, 512], F32, tag="aux")
            nc.tensor.matmul(mp_ps[:, 0:OUT], embT_t[:], wt_t[:],
                             start=True, stop=True)
            mprime = cpool.tile([VOCAB, OUT], BF16)
            nc.scalar.copy(mprime[:], mp_ps[:, 0:OUT])

            # One-hot: onehot[v, p] = (x[p] == v), built in 4-tile chunks
            oh = ohpool.tile([VOCAB, LC], BF16)
            CH = 4 * 128 * STRIDE                      # 4096 positions
            for c0 in range(0, LC, CH):
                c1 = min(LC, c0 + CH)
                nc.vector.tensor_scalar(
                    oh[:, c0:c1], xb_t[:, c0:c1], iota_t[:], None,
                    mybir.AluOpType.is_equal)
            oh3 = oh[:].rearrange("p (w k) -> p w k", k=STRIDE)

            for t in range(N_TILES):
                rows = min(128, NW - t * 128)
                w0, w1 = t * 128, t * 128 + rows
                # k0..k5 -> 6 banks of P; k6 -> a rotating aux bank
                P = pmain.tile([128, 6, 512], F32, tag="p6")
                p6 = paux.tile([128, 512], F32, tag="aux")
                for k in range(6):
                    nc.tensor.matmul(P[0:rows, k, 0:OUT],
                                     oh3[0:VOCAB, w0:w1, k], mprime[:],
                                     start=True, stop=True)
                nc.tensor.matmul(p6[0:rows, 0:OUT],
                                 oh3[0:VOCAB, w0:w1, 6], mprime[:],
                                 start=True, stop=True)

                # Escape: ACT batch-copies banks 1..5 and k6 -> bf16 SBUF.
                cc = wpool.tile([128, 5, OUT], BF16, tag="cc")
                nc.scalar.copy(cc[0:rows, :, :], P[0:rows, 1:6, 0:OUT])
                S = wpool.tile([128, 4, OUT], BF16, tag="S")
                nc.scalar.copy(S[0:rows, 1, :], p6[0:rows, 0:OUT])

                # DVE tree (batched): m1 = max(k0, k1); q2 = (k2,k3),(k4,k5)
                nc.vector.tensor_max(S[0:rows, 0, :], P[0:rows, 0, 0:OUT],
                                     cc[0:rows, 0, :])
                nc.vector.tensor_max(S[0:rows, 2:4, :],
                                     cc[0:rows, 1:5:2, :],
                                     cc[0:rows, 2:5:2, :])
                rr = wpool.tile([128, 2, OUT], BF16, tag="rr")
                nc.vector.tensor_max(rr[0:rows, :, :], S[0:rows, 0:4:2, :],
                                     S[0:rows, 1:4:2, :])
                res = wpool.tile([128, OUT], F32, tag="res")
                nc.vector.tensor_max(res[0:rows, :], rr[0:rows, 0, :],
                                     rr[0:rows, 1, :])
                nc.sync.dma_start(out_dram[w0:w1, :], res[0:rows, :])

    nc.compile()
    return nc


def _host_inputs(x, emb_table, conv_w, conv_b):
    """Build per-core device input tensors (layout/dtype prep only)."""
    bf16 = ml_dtypes.bfloat16

    embT_aug = np.zeros((KDIM, VOCAB), bf16)
    embT_aug[:EMB, :] = emb_table.T.astype(bf16)
    embT_aug[EMB, :] = bf16(1.0)                     # ones row -> bias

    wt_aug = np.zeros((KDIM, OUT), bf16)
    wt_aug[:EMB, :] = conv_w.T.astype(bf16)
    wt_aug[EMB, :] = conv_b.astype(bf16)

    iota = np.arange(VOCAB, dtype=np.float32).reshape(VOCAB, 1)

    xbs = []
    for c in range(N_CORES):
        xc = x[c * B_CORE:(c + 1) * B_CORE].reshape(-1)   # [12800]
        xbs.append(np.broadcast_to(xc.astype(bf16), (VOCAB, LC)).copy())

    return embT_aug, wt_aug, iota, xbs


def _expected_wordidx():
    pattern = np.concatenate([np.ones(WORD_LEN, np.int64), np.zeros(1, np.int64)])
    return np.tile(pattern, NUM_WORDS)[None, :].repeat(B, axis=0)


def _host_fallback(x, wordidx, emb_table, conv_w, conv_b):
    """Exact reference math on host (only for unexpected wordidx layouts)."""
    e = emb_table[x]
    h = np.einsum('blc,oc->blo', e, conv_w) + conv_b
    bi = (wordidx == 0).astype(np.int64)
    word_id = np.cumsum(bi, axis=1) - bi
    word_id = np.minimum(word_id, NUM_WORDS - 1)
    valid = wordidx > 0
    out = np.full((B, NUM_WORDS, OUT), -np.inf, np.float32)
    for b in range(B):
        for w in range(NUM_WORDS):
            m = valid[b] & (word_id[b] == w)
            if m.any():
                out[b, w] = h[b, m].max(axis=0)
    return out


def kernel(x, wordidx, emb_table, conv_w, conv_b):
    global LAST_RESULTS
    x = np.asarray(x)
    wordidx = np.asarray(wordidx)
    emb_table = np.asarray(emb_table, np.float32)
    conv_w = np.asarray(conv_w, np.float32)
    conv_b = np.asarray(conv_b, np.float32)

    if not np.array_equal(wordidx.astype(np.int64), _expected_wordidx()):
        return _host_fallback(x.astype(np.int64), wordidx.astype(np.int64),
                              emb_table, conv_w, conv_b)

    embT_aug, wt_aug, iota, xbs = _host_inputs(
        x.astype(np.int64), emb_table, conv_w, conv_b)

    nc = _build_program()
    in_maps = [
        {"xb": xbs[c], "iota": iota, "embT_aug": embT_aug, "wt_aug": wt_aug}
        for c in range(N_CORES)
    ]
    res = bass_utils.run_bass_kernel_spmd(nc, in_maps,
                                          core_ids=list(range(N_CORES)))
    LAST_RESULTS = res
    out = np.concatenate([res.results[c]["out"] for c in range(N_CORES)], axis=0)
    return out.reshape(B, NUM_WORDS, OUT).astype(np.float32)


# revision 5
# speedup vs baseline: 1.3700x; 1.3700x over previous
"""Trainium2 Bass kernel for nn_CNN_CharEmb.

Computation: character embeddings -> pointwise conv (per-position linear) ->
ragged per-word max-pool over the 7 chars of each word:

  out[b, w, :] = max_{k=0..6} ( emb[x[b, 8w+k]] @ conv_w.T + conv_b )

Device strategy (8 NeuronCores, batch-sharded, 4 rows/core):
  1. Fused table M' = emb @ conv_w.T + conv_b  [70, 300] built on device by
     one matmul (the ones-row/bias-row augmentation folds the bias in), so
     h[pos] = M'[x[pos]] and the embedding+conv collapse into a row-select.
  2. The row-select is a one-hot matmul: a single DVE is_equal against an
     iota turns x (broadcast across 70 partitions) into onehot [70, L] bf16;
     then h_k tile = onehot_slice.T @ M' on the PE (no per-index DMA work).
  3. Per word-tile (128 words), 7 matmuls (char slots k=0..6, stride-8
     column slices of the onehot) land in 7 PSUM banks; a max tree split
     across DVE/ACT/GPSIMD reduces them and the f32 result DMAs out.

`wordidx` is the fixed 7-chars+boundary pattern of the reference setup;
anything else falls back to an exact host computation.
"""

import numpy as np
import ml_dtypes

import concourse.bacc as bacc
import concourse.mybir as mybir
import concourse.tile as tile
from concourse import bass_utils

# Problem shape (hardcoded per contract)
B = 32
WORD_LEN = 7
NUM_WORDS = 400
STRIDE = WORD_LEN + 1            # 8
L = NUM_WORDS * STRIDE           # 3200
EMB = 100
OUT = 300
VOCAB = 70

N_CORES = 8
B_CORE = B // N_CORES            # 4 batch rows per core
NW = B_CORE * NUM_WORDS          # 1600 words per core
LC = B_CORE * L                  # 12800 positions per core
N_TILES = (NW + 127) // 128      # 13 word-tiles (last one 64 words)
KDIM = EMB + 1                   # 101: emb + ones/bias row

BF16 = mybir.dt.bfloat16
F32 = mybir.dt.float32

LAST_RESULTS = None  # stashed BassKernelResults for the test harness


def _build_program():
    nc = bacc.Bacc("TRN2", target_bir_lowering=False, debug=False,
                   num_devices=N_CORES)

    xb_dram = nc.dram_tensor("xb", [VOCAB, LC], BF16, kind="ExternalInput")
    iota_dram = nc.dram_tensor("iota", [VOCAB, 1], F32, kind="ExternalInput")
    embT_dram = nc.dram_tensor("embT_aug", [KDIM, VOCAB], BF16,
                               kind="ExternalInput")
    wt_dram = nc.dram_tensor("wt_aug", [KDIM, OUT], BF16, kind="ExternalInput")
    out_dram = nc.dram_tensor("out", [NW, OUT], F32, kind="ExternalOutput")

    with tile.TileContext(nc) as tc:
        with (
            tc.tile_pool(name="const", bufs=1) as cpool,
            tc.tile_pool(name="oh", bufs=1) as ohpool,
            tc.tile_pool(name="work", bufs=3) as wpool,
            tc.tile_pool(name="pa3", bufs=1, space="PSUM") as papool,
            tc.tile_pool(name="pb3", bufs=1, space="PSUM") as pbpool,
            tc.tile_pool(name="paux", bufs=2, space="PSUM") as paux,
        ):
            embT_t = cpool.tile([KDIM, VOCAB], BF16)
            wt_t = cpool.tile([KDIM, OUT], BF16)
            iota_t = cpool.tile([VOCAB, 1], F32)
            xb_t = cpool.tile([VOCAB, LC], BF16)
            nc.sync.dma_start(embT_t[:], embT_dram[:])
            nc.sync.dma_start(wt_t[:], wt_dram[:])
            nc.sync.dma_start(iota_t[:], iota_dram[:])

            # Fused table M' = emb @ W.T + b   [70, 300] bf16
            mp_ps = paux.tile([VOCAB, 512], F32, tag="aux")
            nc.tensor.matmul(mp_ps[:, 0:OUT], embT_t[:], wt_t[:],
                             start=True, stop=True)
            mprime = cpool.tile([VOCAB, OUT], BF16)
            nc.scalar.copy(mprime[:], mp_ps[:, 0:OUT])

            # One-hot: onehot[v, p] = (x[p] == v); xb load + is_equal are
            # chunked (4 tiles each) and interleaved with the word-tile loop.
            oh = ohpool.tile([VOCAB, LC], BF16)
            oh3 = oh[:].rearrange("p (w k) -> p w k", k=STRIDE)
            CH = 4 * 128 * STRIDE                      # 4096 positions

            def emit_oh_chunk(c):
                c0, c1 = c * CH, min(LC, (c + 1) * CH)
                nc.sync.dma_start(xb_t[:, c0:c1], xb_dram[:, c0:c1])
                nc.vector.tensor_scalar(
                    oh[:, c0:c1], xb_t[:, c0:c1], iota_t[:], None,
                    mybir.AluOpType.is_equal)

            for t in range(N_TILES):
                if t % 4 == 0:
                    emit_oh_chunk(t // 4)
                rows = min(128, NW - t * 128)
                w0, w1 = t * 128, t * 128 + rows
                # k0,1,2 -> A banks; k3,4,5 -> B banks; k6 -> rotating aux
                A = papool.tile([128, 3, 512], F32, tag="pa")
                Bp = pbpool.tile([128, 3, 512], F32, tag="pb")
                p6 = paux.tile([128, 512], F32, tag="aux")
                for k in range(3):
                    nc.tensor.matmul(A[0:rows, k, 0:OUT],
                                     oh3[0:VOCAB, w0:w1, k], mprime[:],
                                     start=True, stop=True)
                for k in range(3):
                    nc.tensor.matmul(Bp[0:rows, k, 0:OUT],
                                     oh3[0:VOCAB, w0:w1, 3 + k], mprime[:],
                                     start=True, stop=True)
                nc.tensor.matmul(p6[0:rows, 0:OUT],
                                 oh3[0:VOCAB, w0:w1, 6], mprime[:],
                                 start=True, stop=True)

                # Escape + max tree split across ACT (copies) and DVE (maxes).
                S = wpool.tile([128, 5, OUT], BF16, tag="S")
                nc.scalar.copy(S[0:rows, 0:2, :], A[0:rows, 1:3, 0:OUT])
                nc.scalar.copy(S[0:rows, 2:4, :], Bp[0:rows, 1:3, 0:OUT])
                nc.scalar.copy(S[0:rows, 4, :], p6[0:rows, 0:OUT])

                T = wpool.tile([128, 2, OUT], BF16, tag="T")
                nc.vector.tensor_max(T[0:rows, 0, :], A[0:rows, 0, 0:OUT],
                                     S[0:rows, 0, :])
                nc.vector.tensor_max(T[0:rows, 1, :], Bp[0:rows, 0, 0:OUT],
                                     S[0:rows, 2, :])
                U = wpool.tile([128, 2, OUT], BF16, tag="U")
                nc.vector.tensor_max(U[0:rows, :, :], T[0:rows, 0:2, :],
                                     S[0:rows, 1:4:2, :])
                V = wpool.tile([128, OUT], BF16, tag="V")
                nc.vector.tensor_max(V[0:rows, :], U[0:rows, 0, :],
                                     U[0:rows, 1, :])
                res = wpool.tile([128, OUT], BF16, tag="res")
                nc.vector.tensor_max(res[0:rows, :], V[0:rows, :],
                                     S[0:rows, 4, :])
                # casting DMA (SWDGE): bf16 -> f32 on the way out
                nc.gpsimd.dma_start(out_dram[w0:w1, :], res[0:rows, :])

    nc.compile()
    return nc


def _host_inputs(x, emb_table, conv_w, conv_b):
    """Build per-core device input tensors (layout/dtype prep only)."""
    bf16 = ml_dtypes.bfloat16

    embT_aug = np.zeros((KDIM, VOCAB), bf16)
    embT_aug[:EMB, :] = emb_table.T.astype(bf16)
    embT_aug[EMB, :] = bf16(1.0)                     # ones row -> bias

    wt_aug = np.zeros((KDIM, OUT), bf16)
    wt_aug[:EMB, :] = conv_w.T.astype(bf16)
    wt_aug[EMB, :] = conv_b.astype(bf16)

    iota = np.arange(VOCAB, dtype=np.float32).reshape(VOCAB, 1)

    xbs = []
    for c in range(N_CORES):
        xc = x[c * B_CORE:(c + 1) * B_CORE].reshape(-1)   # [12800]
        xbs.append(np.broadcast_to(xc.astype(bf16), (VOCAB, LC)).copy())

    return embT_aug, wt_aug, iota, xbs


def _expected_wordidx():
    pattern = np.concatenate([np.ones(WORD_LEN, np.int64), np.zeros(1, np.int64)])
    return np.tile(pattern, NUM_WORDS)[None, :].repeat(B, axis=0)


def _host_fallback(x, wordidx, emb_table, conv_w, conv_b):
    """Exact reference math on host (only for unexpected wordidx layouts)."""
    e = emb_table[x]
    h = np.einsum('blc,oc->blo', e, conv_w) + conv_b
    bi = (wordidx == 0).astype(np.int64)
    word_id = np.cumsum(bi, axis=1) - bi
    word_id = np.minimum(word_id, NUM_WORDS - 1)
    valid = wordidx > 0
    out = np.full((B, NUM_WORDS, OUT), -np.inf, np.float32)
    for b in range(B):
        for w in range(NUM_WORDS):
            m = valid[b] & (word_id[b] == w)
            if m.any():
                out[b, w] = h[b, m].max(axis=0)
    return out


def kernel(x, wordidx, emb_table, conv_w, conv_b):
    global LAST_RESULTS
    x = np.asarray(x)
    wordidx = np.asarray(wordidx)
    emb_table = np.asarray(emb_table, np.float32)
    conv_w = np.asarray(conv_w, np.float32)
    conv_b = np.asarray(conv_b, np.float32)

    if not np.array_equal(wordidx.astype(np.int64), _expected_wordidx()):
        return _host_fallback(x.astype(np.int64), wordidx.astype(np.int64),
                              emb_table, conv_w, conv_b)

    embT_aug, wt_aug, iota, xbs = _host_inputs(
        x.astype(np.int64), emb_table, conv_w, conv_b)

    nc = _build_program()
    in_maps = [
        {"xb": xbs[c], "iota": iota, "embT_aug": embT_aug, "wt_aug": wt_aug}
        for c in range(N_CORES)
    ]
    res = bass_utils.run_bass_kernel_spmd(nc, in_maps,
                                          core_ids=list(range(N_CORES)))
    LAST_RESULTS = res
    out = np.concatenate([res.results[c]["out"] for c in range(N_CORES)], axis=0)
    return out.reshape(B, NUM_WORDS, OUT).astype(np.float32)
